# revision 5
# baseline (speedup 1.0000x reference)
"""Trainium2 Bass kernel for nn_BaseMessageModule (GNN message passing).

Math (per edge e with destination atom j = pairlist[1][e]):
    proto[e]  = f_ij_cutoff[e] * features[j]            # [F]
    radial[j]+= proto[e]
    vec[j]   += u[e] * sum(proto[e]),  u = r_ij/|r_ij|
    vector_norms = |vec|

Distribution strategy (8 cores): shard by DESTINATION atom range (12500
atoms/core).  Each edge belongs to exactly one core, so there is no
cross-core reduction at all.  On the host, each core's edges are sorted by
destination atom and bucketed into fixed-capacity per-atom-block slots
(128-atom blocks x CPB*128 edge slots), padded with zero edges.

Device kernel per core (all layouts prepared host-side):
  - features rows gathered per edge via gpsimd dma_gather (int16 local ids)
  - per 128-edge chunk: one-hot S = (iota == idx) via DVE tensor_scalar,
    proto + s = sum(f*g) in ONE fused tensor_tensor_reduce,
    y = u*s via tensor_scalar, then a single PE matmul
    S^T @ [proto | y] accumulated in PSUM over the block's chunks,
    yielding the block's radial rows and vec sums directly.
  - epilogue per block: PSUM -> SBUF copy (ScalarE), vector norm, DMA out.
"""

import os
from contextlib import ExitStack

import ml_dtypes
import numpy as np

import concourse.bacc as bacc
import concourse.tile as tile
from concourse import mybir
from concourse.bass_utils import run_bass_kernel_spmd
from concourse.library_config import mlp

F32 = mybir.dt.float32
BF16 = mybir.dt.bfloat16
I16 = mybir.dt.int16

# ---- problem constants (hardcoded per contest contract) ----
N_ATOMS = 100_000
N_PAIRS = 1_000_000
NF = 128
P = 128
NCORES = 8
APC = N_ATOMS // NCORES          # atoms per core = 12500
NBLK = (APC + P - 1) // P        # atom blocks per core = 98
NATOM_PAD = NBLK * P             # 12544
CPB = 12                         # chunks (of 128 edge slots) per block
SPB = CPB * P                    # edge slots per block = 1536
NCHUNK = NBLK * CPB              # 1176
SLOTS = NCHUNK * P               # 150528
GRP_BLOCKS = 7                   # blocks per dma_gather group
NGRP = NBLK // GRP_BLOCKS        # 14
GRP_CHUNKS = GRP_BLOCKS * CPB    # 84
GRP_IDX = GRP_CHUNKS * P         # 10752

_IOTA = np.ascontiguousarray(
    np.broadcast_to(np.arange(NF, dtype=np.float32), (P, NF))
).astype(ml_dtypes.bfloat16)

LAST_RESULTS = None              # BassKernelResults of the last kernel() call
_NC_CACHE = {}


def _build_nc(cpb=CPB):
    nblk, grp_blocks = NBLK, GRP_BLOCKS
    ngrp = nblk // grp_blocks
    nchunk = nblk * cpb
    grp_chunks = grp_blocks * cpb
    grp_idx = grp_chunks * P

    nc = bacc.Bacc("TRN2", target_bir_lowering=False, debug=False,
                   num_devices=NCORES)
    t_feats = nc.dram_tensor("feats", [NATOM_PAD, NF], BF16,
                             kind="ExternalInput")
    t_f = nc.dram_tensor("f", [P, nchunk, NF], BF16, kind="ExternalInput")
    t_u = nc.dram_tensor("u", [P, nchunk, 3], F32, kind="ExternalInput")
    t_ic = nc.dram_tensor("ic", [P, nchunk], F32, kind="ExternalInput")
    t_gi = nc.dram_tensor("gi", [P, ngrp, grp_idx // 16], I16,
                          kind="ExternalInput")
    t_iota = nc.dram_tensor("iota", [P, NF], BF16, kind="ExternalInput")
    t_radial = nc.dram_tensor("radial", [nblk, P, NF], F32,
                              kind="ExternalOutput")
    t_norms = nc.dram_tensor("norms", [P, nblk], F32, kind="ExternalOutput")

    feats, f_ap, u_ap, ic_ap, gi_ap, iota_ap, radial_ap, norms_ap = (
        t.ap() for t in (t_feats, t_f, t_u, t_ic, t_gi, t_iota, t_radial,
                         t_norms))

    with tile.TileContext(nc) as tc, ExitStack() as ctx:
        GC = grp_chunks
        singles = ctx.enter_context(tc.tile_pool(name="singles", bufs=1))
        fpool = ctx.enter_context(tc.tile_pool(name="fpool", bufs=2))
        gpool = ctx.enter_context(tc.tile_pool(name="gpool", bufs=2))
        upool = ctx.enter_context(tc.tile_pool(name="upool", bufs=2))
        icpool = ctx.enter_context(tc.tile_pool(name="icpool", bufs=2))
        gipool = ctx.enter_context(tc.tile_pool(name="gipool", bufs=2))
        spool = ctx.enter_context(tc.tile_pool(name="spool", bufs=4))
        rpool = ctx.enter_context(tc.tile_pool(name="rpool", bufs=4))
        wpool = ctx.enter_context(tc.tile_pool(name="wpool", bufs=4))
        radpool = ctx.enter_context(tc.tile_pool(name="radpool", bufs=3))
        psum = ctx.enter_context(tc.tile_pool(name="psum", bufs=2,
                                              space="PSUM"))

        nc.gpsimd.load_library(mlp)

        iota_t = singles.tile([P, NF], BF16)
        nc.sync.dma_start(iota_t[:], iota_ap[:])
        norms_t = singles.tile([P, nblk], F32)

        for g in range(ngrp):
            sl = slice(g * GC, (g + 1) * GC)
            f_t = fpool.tile([P, GC, NF], BF16)
            nc.sync.dma_start(f_t[:], f_ap[:, sl, :])
            u_t = upool.tile([P, GC, 3], F32)
            nc.sync.dma_start(u_t[:], u_ap[:, sl, :])
            ic_t = icpool.tile([P, GC], F32)
            nc.sync.dma_start(ic_t[:], ic_ap[:, sl])
            gi_t = gipool.tile([P, grp_idx // 16], I16)
            nc.sync.dma_start(gi_t[:], gi_ap[:, g, :])
            g_t = gpool.tile([P, GC, NF], BF16)
            nc.gpsimd.dma_gather(g_t[:], feats[:], gi_t[:], grp_idx, grp_idx,
                                 NF, single_packet=False)

            for bb in range(grp_blocks):
                b = g * grp_blocks + bb
                acc = psum.tile([P, 132], F32)
                scols = wpool.tile([P, cpb], F32, tag="scols")
                for c in range(cpb):
                    s = bb * cpb + c
                    S_t = spool.tile([P, NF], BF16)
                    nc.vector.tensor_scalar(S_t[:], iota_t[:],
                                            ic_t[:, s:s + 1], None,
                                            mybir.AluOpType.is_equal)
                    rhs_t = rpool.tile([P, 132], BF16)
                    nc.vector.scalar_tensor_tensor(
                        out=rhs_t[:, 0:NF],
                        in0=f_t[:, s, :],
                        scalar=1.0,
                        in1=g_t[:, s, :],
                        op0=mybir.AluOpType.mult,
                        op1=mybir.AluOpType.mult,
                        accum_out=scols[:, c:c + 1],
                    )
                    nc.vector.tensor_scalar(rhs_t[:, NF:NF + 3], u_t[:, s, :],
                                            scols[:, c:c + 1], None,
                                            mybir.AluOpType.mult)
                    nc.tensor.matmul(acc[:, 0:NF + 3], lhsT=S_t[:],
                                     rhs=rhs_t[:, 0:NF + 3], start=(c == 0),
                                     stop=(c == cpb - 1))
                sb_t = radpool.tile([P, NF + 3], F32)
                nc.scalar.copy(sb_t[:], acc[:, 0:NF + 3])
                nc.sync.dma_start(radial_ap[b], sb_t[:, 0:NF])
                n2 = wpool.tile([P, 1], F32, tag="n2")
                v2 = wpool.tile([P, 3], F32, tag="v2")
                nc.vector.scalar_tensor_tensor(
                    out=v2[:], in0=sb_t[:, NF:NF + 3], scalar=1.0,
                    in1=sb_t[:, NF:NF + 3], op0=mybir.AluOpType.mult,
                    op1=mybir.AluOpType.mult, accum_out=n2[:])
                nc.scalar.activation(norms_t[:, b:b + 1], n2[:],
                                     mybir.ActivationFunctionType.Sqrt)
        nc.sync.dma_start(norms_ap[:], norms_t[:])
    nc.compile()
    return nc


def _get_nc(cpb=CPB):
    if cpb not in _NC_CACHE:
        _NC_CACHE[cpb] = _build_nc(cpb)
    return _NC_CACHE[cpb]


def _prep_inputs(features, pairlist, f_ij_cutoff, r_ij, cpb=CPB):
    spb = cpb * P
    nchunk = NBLK * cpb
    slots = nchunk * P
    grp_chunks = GRP_BLOCKS * cpb
    grp_idx = grp_chunks * P

    idx = np.asarray(pairlist)[1].astype(np.int64)
    r = np.asarray(r_ij, np.float32)
    u = r / np.linalg.norm(r, axis=1, keepdims=True)
    f = np.asarray(f_ij_cutoff, np.float32)
    feats = np.asarray(features, np.float32)

    order = np.argsort(idx, kind="stable")
    sidx = idx[order]
    bounds = np.searchsorted(sidx, np.arange(0, N_ATOMS + APC, APC))

    in_maps = []
    max_count = 0
    for c in range(NCORES):
        lo, hi = bounds[c], bounds[c + 1]
        eidx = order[lo:hi]
        la = sidx[lo:hi] - c * APC
        blk = la >> 7
        counts = np.bincount(blk, minlength=NBLK)
        max_count = max(max_count, int(counts.max()))
        if counts.max() > spb:
            return None, max_count
        starts = np.concatenate(([0], np.cumsum(counts[:-1])))
        rank = np.arange(la.size) - starts[blk]
        slot = blk * spb + rank

        ff = np.zeros((slots, NF), ml_dtypes.bfloat16)
        ff[slot] = f[eidx]
        uu = np.zeros((slots, 3), np.float32)
        uu[slot] = u[eidx]
        ic = np.full(slots, 255.0, np.float32)
        ic[slot] = la & 127
        gi = np.zeros(slots, np.int16)
        gi[slot] = la

        featsd = np.zeros((NATOM_PAD, NF), ml_dtypes.bfloat16)
        featsd[:APC] = feats[c * APC:(c + 1) * APC]
        # index i of a group lives at partition i%16, column i//16; the 8
        # Q7 cores each read their own 16-partition window, so replicate 8x.
        gid = np.tile(gi.reshape(NGRP, grp_idx // 16, 16).transpose(2, 0, 1),
                      (8, 1, 1))

        in_maps.append({
            "feats": featsd,
            "f": np.ascontiguousarray(
                ff.reshape(nchunk, P, NF).transpose(1, 0, 2)),
            "u": np.ascontiguousarray(
                uu.reshape(nchunk, P, 3).transpose(1, 0, 2)),
            "ic": np.ascontiguousarray(ic.reshape(nchunk, P).T),
            "gi": gid,
            "iota": _IOTA,
        })
    return in_maps, max_count


def kernel(features, pairlist, f_ij_cutoff, r_ij):
    global LAST_RESULTS
    cpb = CPB
    in_maps, max_count = _prep_inputs(features, pairlist, f_ij_cutoff, r_ij,
                                      cpb)
    while in_maps is None:      # block overflow: grow capacity and recompile
        cpb = -(-max_count // P) + 1
        in_maps, max_count = _prep_inputs(features, pairlist, f_ij_cutoff,
                                          r_ij, cpb)
    nc = _get_nc(cpb)
    trace = bool(int(os.environ.get("GNN_TRACE", "0")))
    res = run_bass_kernel_spmd(nc, in_maps, core_ids=list(range(NCORES)),
                               trace=trace)
    LAST_RESULTS = res

    radial = np.empty((N_ATOMS, NF), np.float32)
    norms = np.empty(N_ATOMS, np.float32)
    for c, out in enumerate(res.results):
        rad = np.asarray(out["radial"], np.float32)
        radial[c * APC:(c + 1) * APC] = rad.reshape(NATOM_PAD, NF)[:APC]
        nr = np.asarray(out["norms"], np.float32)
        norms[c * APC:(c + 1) * APC] = nr.T.reshape(NATOM_PAD)[:APC]
    return radial, norms
